# revision 18
# baseline (speedup 1.0000x reference)
"""Trainium2 Bass kernel for nn_LocalFWLNet (gnn_message_passing).

Self-contained: host front-end (tiny GCN/MLP/scatter) in numpy, the heavy
[n,n,d] einsum + mlp3 on 8 NeuronCores via bass/Tile, GraphNorm moments +
final pair gather + linear on host (device returns pre-norm z plus its
masked sums, which is exact because z is zero off-support by construction).

Device sharding: 2D grid (CI=2 i-blocks x CJ=4 j-blocks) over the dense
[n,n,d] pair tensors, fp8 inputs. Each core contracts its full-k strip:
    C[i_blk, j_blk, d] = sum_k Xd[i_blk, k, d] * Md[k, j_blk, d]
with the adjacency matrix rolled in as a 33rd channel (A = A @ I), then
computes z' = C @ W3[:32] + af*W3[32] via PE transposes and a block-diagonal
matmul, accumulates sum(z') / sum(z'^2) on the fly, and streams z' out in
fp16.  GraphNorm mean/var (from the per-core sums), the +b3 bias, ReLU and
the symmetric pair product all happen on host at the 2*P gathered positions.
"""
import json
from contextlib import ExitStack

import numpy as np
import ml_dtypes

import concourse.bass as bass
import concourse.mybir as mybir
import concourse.tile as tile
from concourse.bass_utils import run_bass_kernel_spmd
from concourse.masks import make_identity

# ---------------------------------------------------------------- constants
N = 768          # nodes
H = 32           # hidden dim (d)
EPS = 1e-5

CI, CJ = 2, 4                # core grid over (i, j)
NCORES = CI * CJ
NI, NJ = N // CI, N // CJ    # 384, 192 per-core block
NIT = NI // 128              # 3 i-tiles of 128
DTOT = H + 1                 # 32 channels + adjacency channel
TR = DTOT // 3               # 11 d-triples per i-tile
KT = N // 128                # 6 k-tiles
G = 3                        # j's per transpose group
BD_K = DTOT * G              # 99 blockdiag contraction dim
GRP = NJ // G                # 64 groups per i-tile
KB = 4                       # groups per zmm batch
NB = GRP // KB               # 16 batches per i-tile

F32 = mybir.dt.float32
F16 = mybir.dt.float16
BF16 = mybir.dt.bfloat16
FP8 = mybir.dt.float8e4
FP8_NP = mybir.dt.np(mybir.dt.float8e4)
BF16_NP = ml_dtypes.bfloat16

_CACHE = {}
LAST_RESULTS = None   # set by kernel(); test.py reads exec_time from here
TRACE = [False]       # test.py can flip to enable NTFF tracing


# ------------------------------------------------------- BIR wait splitting
def _split_waits(bir_bytes, maxw=1, maxw_drain=1):
    """walrus rejects instructions with too many sync waits (EventSemaphore
    <=2, Drain ~1). Spill excess waits onto standalone EventSemaphore
    instructions just before the offender on the same engine (same
    instruction stream, so ordering is preserved)."""
    d = json.loads(bir_bytes)
    ctr = 0
    for fn in d.get("functions", []):
        for bb in fn.get("blocks", []):
            out = []
            for inst in bb.get("instructions", []):
                si = inst.get("sync_info")
                waits = si.get("on_wait") if si else None
                lim = maxw_drain if inst.get("opcode") == "Drain" else maxw
                if waits and len(waits) > lim:
                    spill = waits[: len(waits) - lim]
                    si["on_wait"] = waits[len(waits) - lim:]
                    for lo in range(0, len(spill), maxw):
                        ctr += 1
                        out.append({
                            "debug": inst.get("debug"),
                            "engine": inst["engine"],
                            "ins": [],
                            "name": f"wsplit-{ctr}",
                            "opcode": "EventSemaphore",
                            "outs": [],
                            "sync_info": {"on_update": [],
                                          "on_wait": spill[lo: lo + maxw]},
                        })
                out.append(inst)
            bb["instructions"] = out
    return json.dumps(d).encode()


# ------------------------------------------------------------ device kernel
def build_nc():
    nc = bass.Bass()
    # xd[it, d, kp, kt, i2] = Xe[i0 + it*128 + i2, kt*128 + kp, d]
    xd = nc.dram_tensor("xd", [NIT, DTOT, 128, KT, 128], FP8,
                        kind="ExternalInput")
    # md[d, kp, kt, j2] = Me[kt*128 + kp, j0 + j2, d]
    md = nc.dram_tensor("md", [DTOT, 128, KT, NJ], FP8, kind="ExternalInput")
    # wbd[d*3+r, r*32+h] = mlp3_W[d, h]  (zero-padded to 128 cols)
    wbd = nc.dram_tensor("wbd", [BD_K, 128], F16, kind="ExternalInput")
    # zt[r*32+h, it, g, i2] = z'[i0+it*128+i2, j0+g*3+r, h]   (pre-norm)
    zt = nc.dram_tensor("zt", [3 * H, NIT, GRP, 128], F16,
                        kind="ExternalOutput")

    with tile.TileContext(nc) as tc, ExitStack() as ctx:
        def pool(name, bufs, space="SBUF"):
            return ctx.enter_context(
                tc.tile_pool(name=name, bufs=bufs, space=space))

        singles = pool("singles", 1)
        md_sb = singles.tile([128, DTOT, KT, NJ], FP8)
        md_chunks = [(0, 3), (3, 6), (6, 9), (9, 14), (14, 20),
                     (20, 27), (27, 33)]

        def load_md(ci):
            a, b = md_chunks[ci]
            nc.sync.dma_start(
                out=md_sb[:, a:b],
                in_=md[a:b].rearrange("d p k j -> p d k j"))
        load_md(0)
        ident = singles.tile([128, 128], F16)
        make_identity(nc, ident[:])
        wbd_sb = singles.tile([BD_K, 128], F16)
        nc.sync.dma_start(out=wbd_sb, in_=wbd[:])

        # staging: C (and af), interleaved [i2, group, (d*3+r)] per i-tile
        cst = [singles.tile([128, GRP, BD_K], F16, name=f"cst{i}",
                            tag=f"cst{i}") for i in range(NIT)]

        xd_pool = pool("xd", 9)
        psumC = pool("psumC", 4, space="PSUM")
        psumT = pool("psumT", 2, space="PSUM")
        psumZ = pool("psumZ", 2, space="PSUM")
        rhs_pool = pool("rhs", 3)
        zst_pool = pool("zst", 2)

        # d-chunks of 2 (last chunk of 1): each PSUM region fits one bank
        chunks = []
        for it in range(NIT):
            dcur = 0
            while dcur < DTOT:
                w = min(2, DTOT - dcur)
                chunks.append((it, dcur, w))
                dcur += w
        triples = chunks
        xts = {}

        def issue_xt(n):
            if n >= len(triples):
                return
            it_, d0_, w_ = triples[n]
            t = xd_pool.tile([128, w_, KT, 128], FP8)
            nc.sync.dma_start(
                out=t, in_=xd[it_, d0_:d0_ + w_].rearrange(
                    "d p k i -> p d k i"))
            xts[n] = t

        LOOKAHEAD = 8
        # prologue: interleave first xt loads with just-in-time md chunks
        issue_xt(0)
        load_md(1)
        issue_xt(1)
        load_md(2)
        issue_xt(2)
        load_md(3)
        issue_xt(3)
        load_md(4)
        issue_xt(4)
        load_md(5)
        issue_xt(5)
        load_md(6)
        issue_xt(6)
        issue_xt(7)

        from collections import deque

        # phase-2 state machine: batches are emitted one at a time,
        # interleaved with the next i-tile's einsum triples so the
        # in-order PE queue alternates einsum matmuls with transposes
        # and the zmm never stalls on the DVE PSUM->SBUF copy.
        class P2:
            def __init__(self, it):
                self.it = it
                self.zst = zst_pool.tile([3 * H, GRP * 128], F16)
                self.b = 0
                self.pend = deque()

            def transposes(self, b):
                pt = psumT.tile([BD_K, KB * 128], F16)
                for gg in range(KB):
                    g = b * KB + gg
                    nc.tensor.transpose(
                        pt[:, gg * 128:(gg + 1) * 128],
                        cst[self.it][:, g, :],
                        ident[:])
                rhs = rhs_pool.tile([BD_K, KB * 128], F16)
                nc.vector.tensor_copy(out=rhs, in_=pt)
                return rhs

            def zmm(self, b, rhs):
                pz = psumZ.tile([128, KB * 128], F32)
                nc.tensor.matmul(pz, lhsT=wbd_sb, rhs=rhs,
                                 start=True, stop=True)
                zsl = self.zst[:, b * KB * 128:(b + 1) * KB * 128]
                nc.scalar.activation(
                    zsl, pz[0:3 * H, :],
                    mybir.ActivationFunctionType.Copy)
                if b % 2 == 1:
                    c0 = (b - 1) * KB * 128
                    c1 = (b + 1) * KB * 128
                    nc.sync.dma_start(
                        out=zt[:, self.it].rearrange(
                            "p g i -> p (g i)")[:, c0:c1],
                        in_=self.zst[:, c0:c1])

            def emit_one(self):
                if self.b < NB:
                    rhs = self.transposes(self.b)
                    self.pend.append((self.b, rhs))
                    self.b += 1
                    if len(self.pend) > 2:
                        self.zmm(*self.pend.popleft())
                    return True
                if self.pend:
                    self.zmm(*self.pend.popleft())
                    return True
                return False

            def drain(self):
                while self.emit_one():
                    pass

        p2 = None
        for n, (it, d0, w) in enumerate(triples):
            xt = xts.pop(n)
            # 256-col stride keeps each dd region inside one PSUM bank
            pc = psumC.tile([128, w, 256], F32)
            for dd in range(w):
                for ktp in range(KT // 2):
                    nc.tensor.matmul(
                        pc[:, dd, 0:NJ],
                        lhsT=xt[:, dd, 2 * ktp:2 * ktp + 2, :],
                        rhs=md_sb[:, d0 + dd, 2 * ktp:2 * ktp + 2, :],
                        start=(ktp == 0), stop=(ktp == KT // 2 - 1),
                        perf_mode=mybir.MatmulPerfMode.DoubleRow)
            issue_xt(n + LOOKAHEAD)
            dst = cst[it][:, :, 3 * d0:3 * (d0 + w)].rearrange(
                "p g (d r) -> p g d r", r=3)
            src = pc[:, :, 0:NJ].rearrange("p d (g r) -> p g d r", r=3)
            if n % 2 == 0:
                nc.scalar.activation(
                    dst, src, mybir.ActivationFunctionType.Copy)
            else:
                nc.vector.tensor_copy(out=dst, in_=src)
            if p2 is not None:
                p2.emit_one()
            if d0 + w == DTOT:
                if p2 is not None:
                    p2.drain()
                p2 = P2(it)
        p2.drain()

    nc.to_json_bytes = (lambda b: (lambda: b))(
        _split_waits(type(nc).to_json_bytes(nc)))
    return nc


# ----------------------------------------------------------- host front-end
def _front_end(x, ei, pos, emb, gcn_W, gcn_b, mlp1_W, mlp1_b, mlp2_W, mlp2_b):
    h = emb[x].astype(np.float32)
    A = np.zeros((N, N), np.float32)
    A[ei[0], ei[1]] = 1.0
    Ahat = A + np.eye(N, dtype=np.float32)
    dinv = 1.0 / np.sqrt(Ahat.sum(1))
    An = Ahat * dinv[:, None] * dinv[None, :]
    for l in range(gcn_W.shape[0]):
        h = An @ (h @ gcn_W[l]) + gcn_b[l]
        h = h - h.mean(0)
        h = h * (1.0 / np.sqrt((h * h).mean(0) + EPS))
        h = np.maximum(h, 0)
    xx = h[pos[:, 0]] * h[pos[:, 1]]
    val = np.concatenate([h[ei[0]], h[ei[1]]], 1)
    xe = np.maximum(val @ mlp1_W + mlp1_b, 0)
    mul = np.maximum(val @ mlp2_W + mlp2_b, 0)
    flat = ei[0].astype(np.int64) * N + ei[1].astype(np.int64)
    Xd = np.zeros((N * N, H), np.float32)
    Md = np.zeros((N * N, H), np.float32)
    np.add.at(Xd, flat, xe)
    np.add.at(Md, flat, mul)
    Xd = Xd.reshape(N, N, H)
    Md = Md.reshape(N, N, H)
    adj = np.zeros((N, N), bool)
    adj[ei[0], ei[1]] = True
    af = adj.astype(np.float32)
    mask = ((af @ af) > 0) | adj
    return h, xx, Xd, Md, af, mask.astype(np.float32)


def _pack_inputs(Xd, Md, af, mlp3_W):
    """Build per-core input dicts (fp8 einsum operands + blockdiag W3)."""
    Xe = np.empty((N, N, DTOT), np.float32)
    Xe[:, :, :H] = Xd
    Xe[:, :, H] = af
    Me = np.empty((N, N, DTOT), np.float32)
    Me[:, :, :H] = Md
    Me[:, :, H] = np.eye(N, dtype=np.float32)
    # [d, kp, kt, i] / [d, kp, kt, j]
    XdT = np.ascontiguousarray(
        Xe.transpose(2, 1, 0).reshape(DTOT, KT, 128, N).transpose(0, 2, 1, 3)
    ).astype(FP8_NP)
    MdT = np.ascontiguousarray(
        Me.transpose(2, 0, 1).reshape(DTOT, KT, 128, N).transpose(0, 2, 1, 3)
    ).astype(FP8_NP)
    # wbd[d*3+r, r*32+h] = mlp3_W[d, h]
    wbd = np.zeros((BD_K, 128), np.float32)
    for r in range(G):
        wbd[np.arange(DTOT) * 3 + r, r * H:(r + 1) * H] = mlp3_W
    wbd = wbd.astype(np.float16)
    in_maps = []
    for c in range(NCORES):
        ci, cj = divmod(c, CJ)
        i0, j0 = ci * NI, cj * NJ
        # xd[it, d, kp, kt, i2]
        xd_c = np.ascontiguousarray(
            XdT[:, :, :, i0:i0 + NI].reshape(DTOT, 128, KT, NIT, 128)
            .transpose(3, 0, 1, 2, 4))
        md_c = np.ascontiguousarray(MdT[:, :, :, j0:j0 + NJ])
        in_maps.append({"xd": xd_c, "md": md_c, "wbd": wbd})
    return in_maps


def kernel(x, ei, pos, emb, gcn_W, gcn_b, mlp1_W, mlp1_b,
           mlp2_W, mlp2_b, mlp3_W, mlp3_b, lin_W, lin_b):
    global LAST_RESULTS
    x = np.asarray(x)
    ei = np.asarray(ei)
    pos = np.asarray(pos)
    mlp3_W = np.asarray(mlp3_W, np.float32)
    b3 = np.asarray(mlp3_b, np.float64)
    h, xx, Xd, Md, af, m = _front_end(
        x, ei, pos, np.asarray(emb, np.float32),
        np.asarray(gcn_W, np.float32), np.asarray(gcn_b, np.float32),
        np.asarray(mlp1_W, np.float32), np.asarray(mlp1_b, np.float32),
        np.asarray(mlp2_W, np.float32), np.asarray(mlp2_b, np.float32))
    in_maps = _pack_inputs(Xd, Md, af, mlp3_W)
    if "nc" not in _CACHE:
        _CACHE["nc"] = build_nc()
    nc = _CACHE["nc"]
    res = run_bass_kernel_spmd(nc, in_maps, list(range(NCORES)),
                               trace=TRACE[0])
    LAST_RESULTS = res

    # ---- GraphNorm moments from returned z' (exactly 0 off-support)
    S1 = np.zeros(H, np.float64)
    S2 = np.zeros(H, np.float64)
    for c in range(NCORES):
        zc = np.asarray(res.results[c]["zt"]).astype(np.float32)
        zc2 = zc.reshape(G, H, -1)
        S1 += zc2.sum(axis=(0, 2), dtype=np.float64)
        S2 += (zc2.astype(np.float64) ** 2).sum(axis=(0, 2))
    cnt = float(m.sum())
    mean = (S1 + cnt * b3) / cnt
    E2 = (S2 + 2.0 * b3 * S1 + cnt * b3 * b3) / cnt
    var = E2 - mean * mean
    inv = 1.0 / np.sqrt(var + EPS)

    # ---- gather z' at pos pairs (both orientations) from per-core zt
    zts = [np.asarray(res.results[c]["zt"]) for c in range(NCORES)]

    def gather(a, b):
        core = (a // NI) * CJ + (b // NJ)
        il = a % NI
        it, i2 = il // 128, il % 128
        jl = b % NJ
        g, r = jl // G, jl % G
        out = np.empty((len(a), H), np.float32)
        for c in range(NCORES):
            selc = np.nonzero(core == c)[0]
            if len(selc) == 0:
                continue
            rows = (r[selc, None] * H + np.arange(H)[None, :])
            out[selc] = zts[c][rows, it[selc, None], g[selc, None],
                               i2[selc, None]].astype(np.float32)
        return out

    p0 = pos[:, 0].astype(np.int64)
    p1 = pos[:, 1].astype(np.int64)
    z01 = gather(p0, p1).astype(np.float64) + b3
    z10 = gather(p1, p0).astype(np.float64) + b3
    zn0 = np.maximum((z01 - mean) * inv, 0.0)
    zn1 = np.maximum((z10 - mean) * inv, 0.0)
    pair = zn0 * zn1 * m[p0, p1][:, None]
    out = (np.concatenate([pair, xx.astype(np.float64)], 1)
           @ np.asarray(lin_W, np.float64)
           + np.asarray(lin_b, np.float64))
    return out.astype(np.float32)


# revision 19
# speedup vs baseline: 1.0708x; 1.0708x over previous
"""Trainium2 Bass kernel for nn_LocalFWLNet (gnn_message_passing).

Self-contained: host front-end (tiny GCN/MLP/scatter) in numpy, the heavy
[n,n,d] einsum + mlp3 on 8 NeuronCores via bass/Tile, GraphNorm moments +
final pair gather + linear on host (device returns pre-norm z plus its
masked sums, which is exact because z is zero off-support by construction).

Device sharding: 2D grid (CI=2 i-blocks x CJ=4 j-blocks) over the dense
[n,n,d] pair tensors, fp8 inputs. Each core contracts its full-k strip:
    C[i_blk, j_blk, d] = sum_k Xd[i_blk, k, d] * Md[k, j_blk, d]
with the adjacency matrix rolled in as a 33rd channel (A = A @ I), then
computes z' = C @ W3[:32] + af*W3[32] via PE transposes and a block-diagonal
matmul, accumulates sum(z') / sum(z'^2) on the fly, and streams z' out in
fp16.  GraphNorm mean/var (from the per-core sums), the +b3 bias, ReLU and
the symmetric pair product all happen on host at the 2*P gathered positions.
"""
import json
from contextlib import ExitStack

import numpy as np
import ml_dtypes

import concourse.bass as bass
import concourse.mybir as mybir
import concourse.tile as tile
from concourse.bass_utils import run_bass_kernel_spmd
from concourse.masks import make_identity

# ---------------------------------------------------------------- constants
N = 768          # nodes
H = 32           # hidden dim (d)
EPS = 1e-5

CI, CJ = 2, 4                # core grid over (i, j)
NCORES = CI * CJ
NI, NJ = N // CI, N // CJ    # 384, 192 per-core block
NIT = NI // 128              # 3 i-tiles of 128
DTOT = H + 1                 # 32 channels + adjacency channel
TR = DTOT // 3               # 11 d-triples per i-tile
KT = N // 128                # 6 k-tiles
G = 3                        # j's per transpose group
BD_K = DTOT * G              # 99 blockdiag contraction dim
GRP = NJ // G                # 64 groups per i-tile
KB = 4                       # groups per zmm batch
NB = GRP // KB               # 16 batches per i-tile

F32 = mybir.dt.float32
F16 = mybir.dt.float16
BF16 = mybir.dt.bfloat16
FP8 = mybir.dt.float8e4
FP8_NP = mybir.dt.np(mybir.dt.float8e4)
BF16_NP = ml_dtypes.bfloat16

_CACHE = {}
LAST_RESULTS = None   # set by kernel(); test.py reads exec_time from here
TRACE = [False]       # test.py can flip to enable NTFF tracing


# ------------------------------------------------------- BIR wait splitting
def _split_waits(bir_bytes, maxw=1, maxw_drain=1):
    """walrus rejects instructions with too many sync waits (EventSemaphore
    <=2, Drain ~1). Spill excess waits onto standalone EventSemaphore
    instructions just before the offender on the same engine (same
    instruction stream, so ordering is preserved)."""
    d = json.loads(bir_bytes)
    ctr = 0
    for fn in d.get("functions", []):
        for bb in fn.get("blocks", []):
            out = []
            for inst in bb.get("instructions", []):
                si = inst.get("sync_info")
                waits = si.get("on_wait") if si else None
                lim = maxw_drain if inst.get("opcode") == "Drain" else maxw
                if waits and len(waits) > lim:
                    spill = waits[: len(waits) - lim]
                    si["on_wait"] = waits[len(waits) - lim:]
                    for lo in range(0, len(spill), maxw):
                        ctr += 1
                        out.append({
                            "debug": inst.get("debug"),
                            "engine": inst["engine"],
                            "ins": [],
                            "name": f"wsplit-{ctr}",
                            "opcode": "EventSemaphore",
                            "outs": [],
                            "sync_info": {"on_update": [],
                                          "on_wait": spill[lo: lo + maxw]},
                        })
                out.append(inst)
            bb["instructions"] = out
    return json.dumps(d).encode()


# ------------------------------------------------------------ device kernel
def build_nc():
    nc = bass.Bass()
    # xd[it, d, kp, kt, i2] = Xe[i0 + it*128 + i2, kt*128 + kp, d]
    xd = nc.dram_tensor("xd", [NIT, DTOT, 128, KT, 128], FP8,
                        kind="ExternalInput")
    # md[d, kp, kt, j2] = Me[kt*128 + kp, j0 + j2, d]
    md = nc.dram_tensor("md", [DTOT, 128, KT, NJ], FP8, kind="ExternalInput")
    # wbd[d*3+r, r*32+h] = mlp3_W[d, h]  (zero-padded to 128 cols)
    wbd = nc.dram_tensor("wbd", [BD_K, 128], F16, kind="ExternalInput")
    # zt[r*32+h, it, g, i2] = z'[i0+it*128+i2, j0+g*3+r, h]   (pre-norm)
    zt = nc.dram_tensor("zt", [3 * H, NIT, GRP, 128], F16,
                        kind="ExternalOutput")

    with tile.TileContext(nc) as tc, ExitStack() as ctx:
        def pool(name, bufs, space="SBUF"):
            return ctx.enter_context(
                tc.tile_pool(name=name, bufs=bufs, space=space))

        singles = pool("singles", 1)
        md_sb = singles.tile([128, DTOT, KT, NJ], FP8)
        md_chunks = [(0, 3), (3, 6), (6, 9), (9, 14), (14, 20),
                     (20, 27), (27, 33)]

        def load_md(ci):
            a, b = md_chunks[ci]
            nc.sync.dma_start(
                out=md_sb[:, a:b],
                in_=md[a:b].rearrange("d p k j -> p d k j"))
        load_md(0)
        ident = singles.tile([128, 128], F16)
        make_identity(nc, ident[:])
        wbd_sb = singles.tile([BD_K, 128], F16)
        nc.sync.dma_start(out=wbd_sb, in_=wbd[:])

        # staging: C (and af), interleaved [i2, group, (d*3+r)] per i-tile
        cst = [singles.tile([128, GRP, BD_K], F16, name=f"cst{i}",
                            tag=f"cst{i}") for i in range(NIT)]

        xd_pool = pool("xd", 7)
        psumC = pool("psumC", 2, space="PSUM")
        psumT = pool("psumT", 2, space="PSUM")
        psumZ = pool("psumZ", 2, space="PSUM")
        rhs_pool = pool("rhs", 3)
        zst_pool = pool("zst", 2)

        triples = [(it, tr) for it in range(NIT) for tr in range(TR)]
        xts = {}

        def issue_xt(n):
            if n >= len(triples):
                return
            it_, tr_ = triples[n]
            t = xd_pool.tile([128, 3, KT, 128], FP8)
            nc.sync.dma_start(
                out=t, in_=xd[it_, 3 * tr_:3 * tr_ + 3].rearrange(
                    "d p k i -> p d k i"))
            xts[n] = t

        LOOKAHEAD = 6
        # prologue: interleave first xt loads with just-in-time md chunks
        issue_xt(0)
        load_md(1)
        issue_xt(1)
        load_md(2)
        issue_xt(2)
        load_md(3)
        issue_xt(3)
        load_md(4)
        issue_xt(4)
        load_md(5)
        issue_xt(5)
        load_md(6)

        def phase2(it):
            # software-pipelined: zmm for batch b-1 issues after the
            # transposes of batch b, so the PE never stalls on the DVE
            # PSUM->SBUF copy feeding the zmm.
            zst = zst_pool.tile([3 * H, GRP * 128], F16)

            def transposes(b):
                pt = psumT.tile([BD_K, KB * 128], F16)
                for gg in range(KB):
                    g = b * KB + gg
                    nc.tensor.transpose(
                        pt[:, gg * 128:(gg + 1) * 128],
                        cst[it][:, g, :],
                        ident[:])
                rhs = rhs_pool.tile([BD_K, KB * 128], F16)
                nc.vector.tensor_copy(out=rhs, in_=pt)
                return rhs

            def zmm(b, rhs):
                pz = psumZ.tile([128, KB * 128], F32)
                nc.tensor.matmul(pz, lhsT=wbd_sb, rhs=rhs,
                                 start=True, stop=True)
                zsl = zst[:, b * KB * 128:(b + 1) * KB * 128]
                nc.scalar.activation(
                    zsl, pz[0:3 * H, :],
                    mybir.ActivationFunctionType.Copy)
                if b % 2 == 1:
                    c0 = (b - 1) * KB * 128
                    c1 = (b + 1) * KB * 128
                    nc.sync.dma_start(
                        out=zt[:, it].rearrange("p g i -> p (g i)")[:, c0:c1],
                        in_=zst[:, c0:c1])

            from collections import deque
            pend = deque()
            for b in range(NB):
                rhs = transposes(b)
                pend.append((b, rhs))
                if len(pend) > 2:
                    zmm(*pend.popleft())
            while pend:
                zmm(*pend.popleft())

        for n, (it, tr) in enumerate(triples):
            d0 = tr * 3
            xt = xts.pop(n)
            # 256-col stride keeps each dd region inside one PSUM bank
            pc = psumC.tile([128, 3, 256], F32)
            for dd in range(3):
                for ktp in range(KT // 2):
                    nc.tensor.matmul(
                        pc[:, dd, 0:NJ],
                        lhsT=xt[:, dd, 2 * ktp:2 * ktp + 2, :],
                        rhs=md_sb[:, d0 + dd, 2 * ktp:2 * ktp + 2, :],
                        start=(ktp == 0), stop=(ktp == KT // 2 - 1),
                        perf_mode=mybir.MatmulPerfMode.DoubleRow)
            issue_xt(n + LOOKAHEAD)
            dst = cst[it][:, :, 3 * d0:3 * d0 + 9].rearrange(
                "p g (d r) -> p g d r", r=3)
            src = pc[:, :, 0:NJ].rearrange("p d (g r) -> p g d r", r=3)
            if tr % 2 == 0:
                nc.scalar.activation(
                    dst, src, mybir.ActivationFunctionType.Copy)
            else:
                nc.vector.tensor_copy(out=dst, in_=src)
            if tr == TR - 1:
                phase2(it)

    nc.to_json_bytes = (lambda b: (lambda: b))(
        _split_waits(type(nc).to_json_bytes(nc)))
    return nc


# ----------------------------------------------------------- host front-end
def _front_end(x, ei, pos, emb, gcn_W, gcn_b, mlp1_W, mlp1_b, mlp2_W, mlp2_b):
    h = emb[x].astype(np.float32)
    A = np.zeros((N, N), np.float32)
    A[ei[0], ei[1]] = 1.0
    Ahat = A + np.eye(N, dtype=np.float32)
    dinv = 1.0 / np.sqrt(Ahat.sum(1))
    An = Ahat * dinv[:, None] * dinv[None, :]
    for l in range(gcn_W.shape[0]):
        h = An @ (h @ gcn_W[l]) + gcn_b[l]
        h = h - h.mean(0)
        h = h * (1.0 / np.sqrt((h * h).mean(0) + EPS))
        h = np.maximum(h, 0)
    xx = h[pos[:, 0]] * h[pos[:, 1]]
    val = np.concatenate([h[ei[0]], h[ei[1]]], 1)
    xe = np.maximum(val @ mlp1_W + mlp1_b, 0)
    mul = np.maximum(val @ mlp2_W + mlp2_b, 0)
    flat = ei[0].astype(np.int64) * N + ei[1].astype(np.int64)
    Xd = np.zeros((N * N, H), np.float32)
    Md = np.zeros((N * N, H), np.float32)
    np.add.at(Xd, flat, xe)
    np.add.at(Md, flat, mul)
    Xd = Xd.reshape(N, N, H)
    Md = Md.reshape(N, N, H)
    adj = np.zeros((N, N), bool)
    adj[ei[0], ei[1]] = True
    af = adj.astype(np.float32)
    mask = ((af @ af) > 0) | adj
    return h, xx, Xd, Md, af, mask.astype(np.float32)


def _pack_inputs(Xd, Md, af, mlp3_W):
    """Build per-core input dicts (fp8 einsum operands + blockdiag W3)."""
    Xe = np.empty((N, N, DTOT), np.float32)
    Xe[:, :, :H] = Xd
    Xe[:, :, H] = af
    Me = np.empty((N, N, DTOT), np.float32)
    Me[:, :, :H] = Md
    Me[:, :, H] = np.eye(N, dtype=np.float32)
    # [d, kp, kt, i] / [d, kp, kt, j]
    XdT = np.ascontiguousarray(
        Xe.transpose(2, 1, 0).reshape(DTOT, KT, 128, N).transpose(0, 2, 1, 3)
    ).astype(FP8_NP)
    MdT = np.ascontiguousarray(
        Me.transpose(2, 0, 1).reshape(DTOT, KT, 128, N).transpose(0, 2, 1, 3)
    ).astype(FP8_NP)
    # wbd[d*3+r, r*32+h] = mlp3_W[d, h]
    wbd = np.zeros((BD_K, 128), np.float32)
    for r in range(G):
        wbd[np.arange(DTOT) * 3 + r, r * H:(r + 1) * H] = mlp3_W
    wbd = wbd.astype(np.float16)
    in_maps = []
    for c in range(NCORES):
        ci, cj = divmod(c, CJ)
        i0, j0 = ci * NI, cj * NJ
        # xd[it, d, kp, kt, i2]
        xd_c = np.ascontiguousarray(
            XdT[:, :, :, i0:i0 + NI].reshape(DTOT, 128, KT, NIT, 128)
            .transpose(3, 0, 1, 2, 4))
        md_c = np.ascontiguousarray(MdT[:, :, :, j0:j0 + NJ])
        in_maps.append({"xd": xd_c, "md": md_c, "wbd": wbd})
    return in_maps


def kernel(x, ei, pos, emb, gcn_W, gcn_b, mlp1_W, mlp1_b,
           mlp2_W, mlp2_b, mlp3_W, mlp3_b, lin_W, lin_b):
    global LAST_RESULTS
    x = np.asarray(x)
    ei = np.asarray(ei)
    pos = np.asarray(pos)
    mlp3_W = np.asarray(mlp3_W, np.float32)
    b3 = np.asarray(mlp3_b, np.float64)
    h, xx, Xd, Md, af, m = _front_end(
        x, ei, pos, np.asarray(emb, np.float32),
        np.asarray(gcn_W, np.float32), np.asarray(gcn_b, np.float32),
        np.asarray(mlp1_W, np.float32), np.asarray(mlp1_b, np.float32),
        np.asarray(mlp2_W, np.float32), np.asarray(mlp2_b, np.float32))
    in_maps = _pack_inputs(Xd, Md, af, mlp3_W)
    if "nc" not in _CACHE:
        _CACHE["nc"] = build_nc()
    nc = _CACHE["nc"]
    res = run_bass_kernel_spmd(nc, in_maps, list(range(NCORES)),
                               trace=TRACE[0])
    LAST_RESULTS = res

    # ---- GraphNorm moments from returned z' (exactly 0 off-support)
    S1 = np.zeros(H, np.float64)
    S2 = np.zeros(H, np.float64)
    for c in range(NCORES):
        zc = np.asarray(res.results[c]["zt"]).astype(np.float32)
        zc2 = zc.reshape(G, H, -1)
        S1 += zc2.sum(axis=(0, 2), dtype=np.float64)
        S2 += (zc2.astype(np.float64) ** 2).sum(axis=(0, 2))
    cnt = float(m.sum())
    mean = (S1 + cnt * b3) / cnt
    E2 = (S2 + 2.0 * b3 * S1 + cnt * b3 * b3) / cnt
    var = E2 - mean * mean
    inv = 1.0 / np.sqrt(var + EPS)

    # ---- gather z' at pos pairs (both orientations) from per-core zt
    zts = [np.asarray(res.results[c]["zt"]) for c in range(NCORES)]

    def gather(a, b):
        core = (a // NI) * CJ + (b // NJ)
        il = a % NI
        it, i2 = il // 128, il % 128
        jl = b % NJ
        g, r = jl // G, jl % G
        out = np.empty((len(a), H), np.float32)
        for c in range(NCORES):
            selc = np.nonzero(core == c)[0]
            if len(selc) == 0:
                continue
            rows = (r[selc, None] * H + np.arange(H)[None, :])
            out[selc] = zts[c][rows, it[selc, None], g[selc, None],
                               i2[selc, None]].astype(np.float32)
        return out

    p0 = pos[:, 0].astype(np.int64)
    p1 = pos[:, 1].astype(np.int64)
    z01 = gather(p0, p1).astype(np.float64) + b3
    z10 = gather(p1, p0).astype(np.float64) + b3
    zn0 = np.maximum((z01 - mean) * inv, 0.0)
    zn1 = np.maximum((z10 - mean) * inv, 0.0)
    pair = zn0 * zn1 * m[p0, p1][:, None]
    out = (np.concatenate([pair, xx.astype(np.float64)], 1)
           @ np.asarray(lin_W, np.float64)
           + np.asarray(lin_b, np.float64))
    return out.astype(np.float32)
